# revision 1
# baseline (speedup 1.0000x reference)
"""Trainium2 Bass kernel for nn_Conv3DSynthesisLayer.

Computes, per sample b (one NeuronCore each, data-parallel over batch B=8):
  styles = w[b] @ (affine_weight / sqrt(512)).T + affine_bias        [Cin]
  wmod   = weight * styles[None,:,None..] ; demod by rsqrt(sumsq)    [Cout,Cin,3,3,3]
  out    = lrelu(conv3d(x[b], wmod, pad=1) + bias) * sqrt(2)         [Cout,32,32,32]

Implementation notes:
  - Conv is 27 shifted fp32r matmuls (K=Cin=128 on partitions) accumulated in
    PSUM per output d-slice, over an h/w zero-padded x laid out per-slice as
    [128, 34*34] in SBUF.  D-boundary taps are skipped (no d padding).
  - x is DMA'd contiguously into an f32 staging tile, then placed (and rounded
    to f32r) into the padded layout by the DVE; halos are zeroed by DVE copies
    from a zero tile (DMA cannot produce f32r-rounded data, DVE can).
  - Demodulation (per-Cout scale) and the lrelu(.)*sqrt(2) epilogue are fused
    into one ScalarE Prelu op per PSUM bank: out = prelu(psum*scale + bias*g).
    (lrelu is positively homogeneous: g*lrelu(z) == lrelu(g*z).)
  - Weight transposes (Cout-major -> Cin-major for matmul lhsT) run on the PE
    via identity transpose; styles modulation is applied by the DVE on the
    PSUM->SBUF copy-back.
  - Main loop is taps-outer over groups of GD d-slices so matmuls that share
    a stationary operand are adjacent.
"""
import sys

sys.path.insert(0, "/opt/trn_rl_repo")

import numpy as np
from contextlib import ExitStack

import concourse.mybir as mybir
import concourse.tile as tile
from concourse import bacc
from concourse.masks import make_identity
from concourse import bass_utils as _bass_utils
from concourse.bass_utils import run_bass_kernel_spmd

# Enable walrus's LDWEIGHTS dedup so consecutive matmuls sharing a stationary
# operand skip the redundant ~191ns weight reload (fp32r matmuls are emitted
# self-loading; the default pipeline pins --enable-ldw-opt=false).
_LDW_OPT = True
if not getattr(_bass_utils, "_ldw_opt_patched", False):
    _orig_run_command = _bass_utils.run_command

    def _run_command_ldw(argv, **kw):
        if _LDW_OPT and isinstance(argv, (list, tuple)):
            argv = ["--enable-ldw-opt=true" if a == "--enable-ldw-opt=false" else a
                    for a in argv]
        return _orig_run_command(argv, **kw)

    _bass_utils.run_command = _run_command_ldw
    _bass_utils._ldw_opt_patched = True

F32 = mybir.dt.float32
F32R = mybir.dt.float32r
AF = mybir.ActivationFunctionType

B, CIN, COUT, R = 8, 128, 128, 32
W_DIM = 512
NTAPS = 27
RP = R + 2  # 34: h/w padded
GAIN = float(np.sqrt(2.0).astype(np.float32))
SLOPE = 0.2
EPS = 1e-8
DBLK = 4  # d-slices per x block
NBLK = R // DBLK
GD = 2  # d-slices per psum group (taps-outer)
NCORES = 8

_cache = {}


def _build():
    nc = bacc.Bacc("TRN2", target_bir_lowering=False, debug=False, num_devices=NCORES)
    x_d = nc.dram_tensor("x", [CIN, R * R * R], F32, kind="ExternalInput").ap()
    wv_d = nc.dram_tensor("wvec", [W_DIM], F32, kind="ExternalInput").ap()
    wt_d = nc.dram_tensor("weight", [COUT, CIN * NTAPS], F32, kind="ExternalInput").ap()
    aw_d = nc.dram_tensor("aw", [CIN, W_DIM], F32, kind="ExternalInput").ap()
    ab_d = nc.dram_tensor("ab", [CIN], F32, kind="ExternalInput").ap()
    bs_d = nc.dram_tensor("bias", [COUT], F32, kind="ExternalInput").ap()
    out_d = nc.dram_tensor("out", [COUT, R * R * R], F32, kind="ExternalOutput").ap()

    ctx = ExitStack()
    with ctx:
        tc = ctx.enter_context(tile.TileContext(nc))
        singles = ctx.enter_context(tc.tile_pool(name="singles", bufs=1))
        xpool = ctx.enter_context(tc.tile_pool(name="xpool", bufs=4))
        stpool = ctx.enter_context(tc.tile_pool(name="stpool", bufs=2))
        obpool = ctx.enter_context(tc.tile_pool(name="obpool", bufs=4))

        # ---- phase A: styles, modulated transposed weights, demod scale ----
        with tc.tile_pool(name="ps_a", bufs=2, space="PSUM") as ps_a:
            ident = singles.tile([128, 128], F32)
            make_identity(nc, ident)

            aw_sb = singles.tile([128, W_DIM], F32)
            nc.sync.dma_start(out=aw_sb, in_=aw_d)
            wv_sb = singles.tile([128, 4], F32)
            nc.sync.dma_start(out=wv_sb, in_=wv_d.rearrange("(c k) -> k c", k=128))
            ab_sb = singles.tile([128, 1], F32)
            nc.sync.dma_start(out=ab_sb, in_=ab_d.rearrange("(p one) -> p one", one=1))
            bs_sb = singles.tile([128, 1], F32)
            nc.sync.dma_start(out=bs_sb, in_=bs_d.rearrange("(p one) -> p one", one=1))
            wnat = singles.tile([128, CIN * NTAPS], F32)
            nc.sync.dma_start(out=wnat, in_=wt_d)

            # affine_weight.T, chunked over the 512-dim: awt[k%128, ci] per chunk
            awt = singles.tile([128, W_DIM], F32)
            for c in range(4):
                paw = ps_a.tile([128, 128], F32, tag="paw", name=f"paw{c}")
                nc.tensor.transpose(paw, aw_sb[:, c * 128:(c + 1) * 128], ident)
                nc.vector.tensor_copy(out=awt[:, c * 128:(c + 1) * 128], in_=paw)

            # styles[ci] = sum_k aw[ci,k] w[k] / sqrt(512) + ab[ci], as [128,1]
            ps_sty = ps_a.tile([128, 1], F32, tag="ps_sty")
            for c in range(4):
                nc.tensor.matmul(ps_sty, lhsT=awt[:, c * 128:(c + 1) * 128],
                                 rhs=wv_sb[:, c:c + 1], start=(c == 0), stop=(c == 3))
            styles = singles.tile([128, 1], F32)
            nc.scalar.activation(out=styles, in_=ps_sty, func=AF.Identity,
                                 bias=ab_sb, scale=1.0 / float(np.sqrt(W_DIM)))

            # per-tap transpose [co,ci] -> [ci,co], modulate by styles[ci];
            # produce both the f32 copy (for sumsq) and the rounded f32r lhsT.
            wnat_t = wnat.rearrange("p (ci t) -> p t ci", t=NTAPS)
            w1f = singles.tile([128, NTAPS * 128], F32)
            w1r = singles.tile([128, NTAPS * 128], F32R)
            for t in range(NTAPS):
                pw = ps_a.tile([128, 128], F32, tag="paw", name=f"pw{t}")
                nc.tensor.transpose(pw, wnat_t[:, t, :], ident)
                nc.vector.tensor_scalar_mul(out=w1f[:, t * 128:(t + 1) * 128],
                                            in0=pw, scalar1=styles)
                nc.vector.tensor_copy(out=w1r[:, t * 128:(t + 1) * 128],
                                      in_=w1f[:, t * 128:(t + 1) * 128])

            # sumsq[co] = sum_{ci,t} w1f[ci, t*128+co]^2  via ones-matmuls
            sq = singles.tile([128, NTAPS * 128], F32)
            nc.vector.tensor_mul(out=sq, in0=w1f, in1=w1f)
            ones_sb = singles.tile([128, 1], F32)
            nc.vector.memset(ones_sb, 1.0)
            ps_dm = ps_a.tile([128, 1], F32, tag="ps_dm")
            for t in range(NTAPS):
                nc.tensor.matmul(ps_dm, lhsT=sq[:, t * 128:(t + 1) * 128],
                                 rhs=ones_sb, start=(t == 0), stop=(t == NTAPS - 1))
            # scale[co] = GAIN * rsqrt(sumsq+EPS) = 1/sqrt(sumsq/G^2 + EPS/G^2)
            eps_sb = singles.tile([128, 1], F32)
            nc.vector.memset(eps_sb, EPS / (GAIN * GAIN))
            sc_tmp = singles.tile([128, 1], F32)
            nc.scalar.activation(out=sc_tmp, in_=ps_dm, func=AF.Sqrt,
                                 bias=eps_sb, scale=1.0 / (GAIN * GAIN))
            scale_sb = singles.tile([128, 1], F32)
            nc.vector.reciprocal(out=scale_sb, in_=sc_tmp)
            bias_g = singles.tile([128, 1], F32)
            nc.scalar.mul(out=bias_g, in_=bs_sb, mul=GAIN)

        pspool = ctx.enter_context(tc.tile_pool(name="pspool", bufs=2, space="PSUM"))

        # ---- phase B: the conv ----
        zrow = singles.tile([128, DBLK * RP], F32)
        nc.vector.memset(zrow, 0.0)
        zview = zrow.rearrange("p (d e) -> p d e", e=RP)

        x_r = x_d.rearrange("p (d hw) -> p d hw", hw=R * R)
        xblocks = [None] * NBLK

        def load_block(blk):
            # contiguous DMA into f32 staging, then DVE place+round into the
            # h/w-padded f32r block tile (halos zeroed by DVE copies).
            stag = stpool.tile([128, DBLK, R * R], F32, tag="stag", name=f"st{blk}")
            nc.sync.dma_start(out=stag, in_=x_r[:, blk * DBLK:(blk + 1) * DBLK, :])
            xb = xpool.tile([128, DBLK, RP, RP], F32R, tag="xb", name=f"xb{blk}")
            nc.vector.tensor_copy(out=xb[:, :, 0, :], in_=zview)
            nc.vector.tensor_copy(out=xb[:, :, RP - 1, :], in_=zview)
            nc.vector.tensor_copy(out=xb[:, :, :, 0], in_=zview)
            nc.vector.tensor_copy(out=xb[:, :, :, RP - 1], in_=zview)
            stv = stag.rearrange("p d (h w) -> p d h w", w=R)
            for si in range(DBLK):
                nc.vector.tensor_copy(out=xb[:, si, 1:R + 1, 1:R + 1],
                                      in_=stv[:, si, :, :])
            xblocks[blk] = xb

        load_block(0)
        if NBLK > 1:
            load_block(1)

        next_blk = 2
        for g in range(R // GD):
            d0 = g * GD
            # prefetch: ensure block containing d0+GD+1 is loaded
            while next_blk < NBLK and (d0 + GD) // DBLK + 1 >= next_blk:
                load_block(next_blk)
                next_blk += 1

            ds = list(range(d0, d0 + GD))
            # per-d valid kd set and first/last tap bookkeeping
            valid = {d: [kd for kd in range(3) if 0 <= d + kd - 1 < R] for d in ds}
            first_t = {d: min(v) * 9 for d, v in valid.items()}
            last_t = {d: max(v) * 9 + 8 for d, v in valid.items()}

            ps = {(dd, hh): pspool.tile([128, 512], F32, tag=f"ps{dd}{hh}",
                                        name=f"ps{d0}_{dd}{hh}")
                  for dd in range(GD) for hh in range(2)}

            for kd in range(3):
                for kh in range(3):
                    for kw in range(3):
                        t = kd * 9 + kh * 3 + kw
                        lhs = w1r[:, t * 128:(t + 1) * 128]
                        for dd, d in enumerate(ds):
                            s = d + kd - 1
                            if not (0 <= s < R):
                                continue
                            xb = xblocks[s // DBLK]
                            si = s % DBLK
                            for hh in range(2):
                                rhs = xb[:, si, 16 * hh + kh: 16 * hh + kh + 16,
                                         kw:kw + 32]
                                nc.tensor.matmul(
                                    ps[(dd, hh)], lhsT=lhs, rhs=rhs,
                                    start=(t == first_t[d]), stop=(t == last_t[d]))

            for dd, d in enumerate(ds):
                ob = obpool.tile([128, 1024], F32, tag="ob", name=f"ob{d}")
                for hh in range(2):
                    nc.scalar.activation(out=ob[:, hh * 512:(hh + 1) * 512],
                                         in_=ps[(dd, hh)], func=AF.Prelu,
                                         bias=bias_g, scale=scale_sb, alpha=SLOPE)
                nc.sync.dma_start(out=out_d[:, d * 1024:(d + 1) * 1024], in_=ob)

    nc.compile()
    return nc


def kernel(**inputs):
    x = np.ascontiguousarray(np.asarray(inputs["x"], dtype=np.float32))
    w = np.ascontiguousarray(np.asarray(inputs["w"], dtype=np.float32))
    weight = np.ascontiguousarray(np.asarray(inputs["weight"], dtype=np.float32))
    aw = np.ascontiguousarray(np.asarray(inputs["affine_weight"], dtype=np.float32))
    ab = np.ascontiguousarray(np.asarray(inputs["affine_bias"], dtype=np.float32))
    bias = np.ascontiguousarray(np.asarray(inputs["bias"], dtype=np.float32))

    if "nc" not in _cache:
        _cache["nc"] = _build()
    nc = _cache["nc"]

    wt2 = weight.reshape(COUT, CIN * NTAPS)
    in_maps = [
        {
            "x": x[b].reshape(CIN, R * R * R),
            "wvec": w[b],
            "weight": wt2,
            "aw": aw,
            "ab": ab,
            "bias": bias,
        }
        for b in range(B)
    ]
    res = run_bass_kernel_spmd(nc, in_maps, list(range(NCORES)))
    out = np.stack([res.results[b]["out"].reshape(COUT, R, R, R) for b in range(B)])
    return out.astype(np.float32)


def run_traced(**inputs):
    """Like kernel(), but also returns the profiled HW exec time in ns."""
    x = np.asarray(inputs["x"], dtype=np.float32)
    w = np.asarray(inputs["w"], dtype=np.float32)
    weight = np.asarray(inputs["weight"], dtype=np.float32)
    aw = np.asarray(inputs["affine_weight"], dtype=np.float32)
    ab = np.asarray(inputs["affine_bias"], dtype=np.float32)
    bias = np.asarray(inputs["bias"], dtype=np.float32)
    if "nc" not in _cache:
        _cache["nc"] = _build()
    nc = _cache["nc"]
    wt2 = weight.reshape(COUT, CIN * NTAPS)
    in_maps = [
        {"x": x[b].reshape(CIN, R * R * R), "wvec": w[b], "weight": wt2,
         "aw": aw, "ab": ab, "bias": bias}
        for b in range(B)
    ]
    res = run_bass_kernel_spmd(nc, in_maps, list(range(NCORES)), trace=True)
    out = np.stack([res.results[b]["out"].reshape(COUT, R, R, R) for b in range(B)])
    return out.astype(np.float32), res.exec_time_ns, res



# revision 10
# speedup vs baseline: 1.5942x; 1.5942x over previous
"""Trainium2 Bass kernel for nn_Conv3DSynthesisLayer.

Computes, per sample b (one NeuronCore each, data-parallel over batch B=8):
  styles = w[b] @ (affine_weight / sqrt(512)).T + affine_bias        [Cin]
  wmod   = weight * styles[None,:,None..] ; demod by rsqrt(sumsq)    [Cout,Cin,3,3,3]
  out    = lrelu(conv3d(x[b], wmod, pad=1) + bias) * sqrt(2)         [Cout,32,32,32]

Algorithm: Winograd F(4,3) along the depth axis (2x MAC reduction vs the
direct method), direct 3x3 over h/w via 9 shifted fp16 matmuls per
component accumulated in PSUM:
  - fp16 (not bf16): the Winograd recombination amplifies rounding ~6x
    on the m5-heavy outputs; bf16's 8-bit mantissa fails the 2e-2 gate,
    fp16's 10-bit passes with margin. Matmul and DVE rates are equal.
  - x is cast+placed into h/w zero-padded fp16 slices [34,34] (ScalarE
    for the startup blocks, GpSimd for the rest so ScalarE stays free
    for PSUM drains mid-stream).
  - Per z-tile (4 output slices), the DVE builds 6 Winograd components
    v_c from input slices 4z-1..4z+4 (B^T: add/sub at 2x fp16 rate,
    exact power-of-2 scalar muls at 4x rate).
  - Weights: fp16 transposes (1 cyc/row), styles modulation folded into
    the ScalarE PSUM->SBUF drain (per-partition scale), G-combined
    along kd into 6 per-component [cin, 9*cout] fp16 lhsT tiles.
    Components 5 (plain copy) and 0 (one scalar mul) are emitted first
    and the conv visits comps in order [5,0,1,2,3,4] so matmuls start
    before the full G-combine finishes.
  - Matmuls: z-tiles processed in pairs; per (pair, comp): 9 taps x
    2 z x 2 h-halves with the tap loop outermost, so 4 consecutive
    matmuls share the stationary operand (one LDWEIGHTS per 4).
  - Demodulation stays folded into the ScalarE epilogue (Prelu with
    per-cout scale); everything up to the activation is linear.
"""
import sys

sys.path.insert(0, "/opt/trn_rl_repo")

import numpy as np
from contextlib import ExitStack

import concourse.mybir as mybir
import concourse.tile as tile
from concourse import bacc
from concourse.masks import make_identity
from concourse.alu_op_type import AluOpType
from concourse.bass_utils import run_bass_kernel_spmd

F32 = mybir.dt.float32
F16 = mybir.dt.float16
AF = mybir.ActivationFunctionType
ADD = AluOpType.add
SUB = AluOpType.subtract

B, CIN, COUT, R = 8, 128, 128, 32
W_DIM = 512
NTAPS = 27
RP = R + 2  # 34: h/w padded
PS = RP * RP  # 1156 padded slice elems
SS = R * R  # 1024 slice elems
GAIN = float(np.sqrt(2.0).astype(np.float32))
SLOPE = 0.2
EPS = 1e-8
ZT = R // 4  # 8 z-tiles (4 output slices each)
NC = 6  # winograd components
NB = 9 * 128  # per-kd weight block columns
NCORES = 8

_cache = {}


def _build():
    nc = bacc.Bacc("TRN2", target_bir_lowering=False, debug=False, num_devices=NCORES)
    x_d = nc.dram_tensor("x", [CIN, R * R * R], F32, kind="ExternalInput").ap()
    wv_d = nc.dram_tensor("wvec", [W_DIM], F32, kind="ExternalInput").ap()
    wt_d = nc.dram_tensor("weight", [COUT, CIN * NTAPS], F32, kind="ExternalInput").ap()
    aw_d = nc.dram_tensor("aw", [CIN, W_DIM], F32, kind="ExternalInput").ap()
    ab_d = nc.dram_tensor("ab", [CIN], F32, kind="ExternalInput").ap()
    bs_d = nc.dram_tensor("bias", [COUT], F32, kind="ExternalInput").ap()
    out_d = nc.dram_tensor("out", [COUT, R * R * R], F32, kind="ExternalOutput").ap()

    TT = nc.vector.tensor_tensor
    TSM = nc.vector.tensor_scalar_mul

    ctx = ExitStack()
    with ctx:
        tc = ctx.enter_context(tile.TileContext(nc))
        singles = ctx.enter_context(tc.tile_pool(name="singles", bufs=1))

        styles = singles.tile([128, 1], F32)
        scale_sb = singles.tile([128, 1], F32)
        bias_g = singles.tile([128, 1], F32)
        wW = [singles.tile([128, NB], F16, name=f"wW{c}") for c in range(NC)]

        # ---- phase A: styles, modulated+G-combined transposed weights ----
        with tc.tile_pool(name="pa", bufs=1) as pa, \
             tc.tile_pool(name="ps_a", bufs=2, space="PSUM") as ps_a:
            ident = pa.tile([128, 128], F32)
            make_identity(nc, ident)

            aw_sb = pa.tile([128, W_DIM], F32)
            nc.sync.dma_start(out=aw_sb, in_=aw_d)
            wv_sb = pa.tile([128, 4], F32)
            nc.sync.dma_start(out=wv_sb, in_=wv_d.rearrange("(c k) -> k c", k=128))
            ab_sb = pa.tile([128, 1], F32)
            nc.sync.dma_start(out=ab_sb, in_=ab_d.rearrange("(p one) -> p one", one=1))
            bs_sb = pa.tile([128, 1], F32)
            nc.sync.dma_start(out=bs_sb, in_=bs_d.rearrange("(p one) -> p one", one=1))
            wnat = pa.tile([128, CIN * NTAPS], F32)
            nc.sync.dma_start(out=wnat, in_=wt_d)

            # affine_weight.T, chunked over the 512-dim
            awt = pa.tile([128, W_DIM], F32)
            for c in range(4):
                paw = ps_a.tile([128, 128], F32, tag="paw", name=f"paw{c}")
                nc.tensor.transpose(paw, aw_sb[:, c * 128:(c + 1) * 128], ident)
                nc.vector.tensor_copy(out=awt[:, c * 128:(c + 1) * 128], in_=paw)

            # styles[ci] = sum_k aw[ci,k] w[k] / sqrt(512) + ab[ci]
            ps_sty = ps_a.tile([128, 1], F32, tag="ps_sty")
            for c in range(4):
                nc.tensor.matmul(ps_sty, lhsT=awt[:, c * 128:(c + 1) * 128],
                                 rhs=wv_sb[:, c:c + 1], start=(c == 0), stop=(c == 3))
            nc.scalar.activation(out=styles, in_=ps_sty, func=AF.Identity,
                                 bias=ab_sb, scale=1.0 / float(np.sqrt(W_DIM)))

            # per-tap transpose [co,ci]->[ci,co] on the PE (f32 psum),
            # styles modulation + fp16 cast folded into the ScalarE drain.
            wnat_t = wnat.rearrange("p (ci t) -> p t ci", t=NTAPS)
            w1h = pa.tile([128, NTAPS * 128], F16)
            for t in range(NTAPS):
                pw = ps_a.tile([128, 128], F32, tag="paw", name=f"pw{t}")
                nc.tensor.transpose(pw, wnat_t[:, t, :], ident)
                nc.scalar.activation(out=w1h[:, t * 128:(t + 1) * 128], in_=pw,
                                     func=AF.Identity, scale=styles)

            # sumsq[co] = sum_{ci,t} w1h^2  via chained ones-matmuls
            sq = pa.tile([128, NTAPS * 128], F16)
            nc.vector.tensor_mul(out=sq, in0=w1h, in1=w1h)
            ones_sb = pa.tile([128, 1], F16)
            nc.vector.memset(ones_sb, 1.0)
            ps_dm = ps_a.tile([128, 1], F32, tag="ps_dm")
            for t in range(NTAPS):
                nc.tensor.matmul(ps_dm, lhsT=sq[:, t * 128:(t + 1) * 128],
                                 rhs=ones_sb, start=(t == 0), stop=(t == NTAPS - 1))
            # scale[co] = GAIN * rsqrt(sumsq+EPS)
            eps_sb = pa.tile([128, 1], F32)
            nc.vector.memset(eps_sb, EPS / (GAIN * GAIN))
            sc_tmp = pa.tile([128, 1], F32)
            nc.scalar.activation(out=sc_tmp, in_=ps_dm, func=AF.Sqrt,
                                 bias=eps_sb, scale=1.0 / (GAIN * GAIN))
            nc.vector.reciprocal(out=scale_sb, in_=sc_tmp)
            nc.scalar.mul(out=bias_g, in_=bs_sb, mul=GAIN)

            # G-combine along kd (fp16, DVE 2x/4x). Comps 5 and 0 first so
            # the conv (order [5,0,1,2,3,4]) can start before the rest.
            # g0=w0/4; g1=-(w0+w1+w2)/6; g2=(w1-w0-w2)/6;
            # g3=(w0+2w1+4w2)/24; g4=(w0-2w1+4w2)/24; g5=w2
            w0 = w1h[:, 0 * NB:1 * NB]
            w1 = w1h[:, 1 * NB:2 * NB]
            w2 = w1h[:, 2 * NB:3 * NB]
            ga = pa.tile([128, NB], F16)
            gb = pa.tile([128, NB], F16)
            gq = pa.tile([128, NB], F16)
            nc.vector.tensor_copy(out=wW[5], in_=w2)
            TSM(out=wW[0], in0=w0, scalar1=0.25)
            TT(out=ga, in0=w0, in1=w2, op=ADD)        # a = w0+w2
            TT(out=gq, in0=ga, in1=w1, op=ADD)        # t1 = a+w1
            TSM(out=wW[1], in0=gq, scalar1=-1.0 / 6.0)
            TT(out=gq, in0=w1, in1=ga, op=SUB)        # u = w1-a
            TSM(out=wW[2], in0=gq, scalar1=1.0 / 6.0)
            TSM(out=ga, in0=w2, scalar1=4.0)          # 4*w2
            TT(out=gb, in0=ga, in1=w0, op=ADD)        # b = w0+4w2
            TSM(out=ga, in0=w1, scalar1=2.0)          # p2 = 2*w1
            TT(out=gq, in0=gb, in1=ga, op=ADD)        # q = b+p2
            TSM(out=wW[3], in0=gq, scalar1=1.0 / 24.0)
            TT(out=gq, in0=gb, in1=ga, op=SUB)        # r = b-p2
            TSM(out=wW[4], in0=gq, scalar1=1.0 / 24.0)

        # ---- phase B pools ----
        zpool = ctx.enter_context(tc.tile_pool(name="zpool", bufs=1))
        stpool = ctx.enter_context(tc.tile_pool(name="stpool", bufs=3))
        xpool = ctx.enter_context(tc.tile_pool(name="xpool", bufs=4))
        vpool = ctx.enter_context(tc.tile_pool(name="vpool", bufs=3))
        scpool = ctx.enter_context(tc.tile_pool(name="scpool", bufs=1))
        mtpool = ctx.enter_context(tc.tile_pool(name="mtpool", bufs=2))
        opool = ctx.enter_context(tc.tile_pool(name="opool", bufs=1))
        ypool = ctx.enter_context(tc.tile_pool(name="ypool", bufs=2))
        obpool = ctx.enter_context(tc.tile_pool(name="obpool", bufs=2))
        pspool = ctx.enter_context(tc.tile_pool(name="pspool", bufs=2, space="PSUM"))

        zslice = zpool.tile([128, RP, RP], F16)
        nc.vector.memset(zslice, 0.0)
        zrow = zpool.tile([128, 4 * RP], F16)
        nc.vector.memset(zrow, 0.0)
        zview = zrow.rearrange("p (d e) -> p d e", e=RP)

        x_r = x_d.rearrange("p (d hw) -> p d hw", hw=SS)
        DBLK = 4
        NBLK = R // DBLK
        xblocks = [None] * NBLK

        def load_block(blk):
            xb = xpool.tile([128, DBLK, RP, RP], F16, tag="xb", name=f"xb{blk}")
            nc.vector.tensor_copy(out=xb[:, :, 0, :], in_=zview)
            nc.vector.tensor_copy(out=xb[:, :, RP - 1, :], in_=zview)
            nc.vector.tensor_copy(out=xb[:, :, :, 0], in_=zview)
            nc.vector.tensor_copy(out=xb[:, :, :, RP - 1], in_=zview)
            for half in range(2):
                stag = stpool.tile([128, 2, SS], F32, tag="stag",
                                   name=f"st{blk}_{half}")
                d0 = blk * DBLK + half * 2
                nc.sync.dma_start(out=stag, in_=x_r[:, d0:d0 + 2, :])
                stv = stag.rearrange("p d (h w) -> p d h w", w=R)
                for si in range(2):
                    dst = xb[:, half * 2 + si, 1:R + 1, 1:R + 1]
                    nc.scalar.activation(out=dst, in_=stv[:, si, :, :],
                                         func=AF.Identity)
            xblocks[blk] = xb

        def xslice(s):
            if s < 0 or s >= R:
                return zslice[:, :, :]
            return xblocks[s // DBLK][:, s % DBLK, :, :]

        def transform(z):
            """B^T input transform for z-tile z -> v tile [128, 6, RP, RP]."""
            x0, x1, x2, x3, x4, x5 = (xslice(4 * z - 1 + k) for k in range(6))
            v = vpool.tile([128, NC, RP, RP], F16, tag="v", name=f"v{z}")
            sc = scpool.tile([128, 4, RP, RP], F16, tag="sc", name=f"sc{z}")
            sA, sB, sC, sD = (sc[:, i] for i in range(4))
            TT(out=sA, in0=x4, in1=x2, op=SUB)            # e = x4-x2
            TT(out=sC, in0=x0, in1=x2, op=SUB)            # g = x0-x2
            TSM(out=sD, in0=sC, scalar1=4.0)              # 4g
            TT(out=v[:, 0], in0=sD, in1=sA, op=ADD)       # v0 = 4g+e
            TT(out=sC, in0=x1, in1=x2, op=ADD)            # s1
            TT(out=sD, in0=x3, in1=x4, op=ADD)            # s2
            TSM(out=sB, in0=sC, scalar1=-4.0)             # -4 s1
            TT(out=v[:, 1], in0=sD, in1=sB, op=ADD)       # v1 = s2-4s1
            TT(out=sC, in0=x1, in1=x2, op=SUB)            # d1
            TT(out=sD, in0=x4, in1=x3, op=SUB)            # d2
            TSM(out=sB, in0=sC, scalar1=4.0)              # 4 d1
            TT(out=v[:, 2], in0=sD, in1=sB, op=ADD)       # v2 = d2+4d1
            TT(out=sB, in0=x3, in1=x1, op=SUB)            # f = x3-x1
            TSM(out=sC, in0=sB, scalar1=2.0)              # 2f
            TT(out=v[:, 3], in0=sC, in1=sA, op=ADD)       # v3 = 2f+e
            TT(out=v[:, 4], in0=sA, in1=sC, op=SUB)       # v4 = e-2f
            TT(out=sC, in0=x5, in1=x3, op=SUB)            # m5 = x5-x3
            TSM(out=sD, in0=sB, scalar1=-4.0)             # -4f
            TT(out=v[:, 5], in0=sD, in1=sC, op=ADD)       # v5 = m5-4f
            return v

        CORDER = [5, 0, 1, 2, 3, 4]

        load_block(0)
        load_block(1)
        next_blk = 2
        vz = [None] * ZT

        for zp in range(ZT // 2):
            z0, z1 = 2 * zp, 2 * zp + 1
            while next_blk < NBLK and (4 * z1 + 4) // DBLK >= next_blk:
                load_block(next_blk)
                next_blk += 1
            vz[z0] = transform(z0)
            vz[z1] = transform(z1)

            mts = [mtpool.tile([128, NC, SS], F16, tag="mt", name=f"mt{z}")
                   for z in (z0, z1)]
            for c in CORDER:
                pst = {}
                for zi in range(2):
                    for hh in range(2):
                        pst[(zi, hh)] = pspool.tile(
                            [128, 512], F32, tag=f"ps{zi}{hh}",
                            name=f"ps{zp}_{c}_{zi}{hh}")
                for kh in range(3):
                    for kw in range(3):
                        j = kh * 3 + kw
                        lhs = wW[c][:, j * 128:(j + 1) * 128]
                        for zi in range(2):
                            v = vz[z0 + zi]
                            for hh in range(2):
                                rhs = v[:, c, 16 * hh + kh: 16 * hh + kh + 16,
                                        kw:kw + 32]
                                nc.tensor.matmul(pst[(zi, hh)], lhsT=lhs,
                                                 rhs=rhs,
                                                 start=(j == 0), stop=(j == 8))
                for zi in range(2):
                    for hh in range(2):
                        nc.scalar.activation(
                            out=mts[zi][:, c, hh * 512:(hh + 1) * 512],
                            in_=pst[(zi, hh)], func=AF.Identity)

            for zi, z in enumerate((z0, z1)):
                mt = mts[zi]
                m0, m1, m2, m3, m4, m5 = (mt[:, c2, :] for c2 in range(6))
                # A^T output transform (DVE, fp16); 3 scratch lanes, no
                # in-place ops:
                #   y0 = m0+m1+m2+m3+m4      y1 = (m1-m2)+2(m3-m4)
                #   y2 = (m1+m2)+4(m3+m4)    y3 = (m1-m2)+8(m3-m4)+m5
                y = ypool.tile([128, 4, SS], F16, tag="y", name=f"y{z}")
                so = opool.tile([128, 3, SS], F16, tag="so", name=f"so{z}")
                oA, oB, oC = (so[:, i] for i in range(3))
                TT(out=oA, in0=m1, in1=m2, op=ADD)        # p = m1+m2
                TT(out=oB, in0=m3, in1=m4, op=ADD)        # r = m3+m4
                TT(out=oC, in0=m0, in1=oA, op=ADD)        # t = m0+p
                TT(out=y[:, 0], in0=oC, in1=oB, op=ADD)   # y0 = t+r
                TSM(out=oC, in0=oB, scalar1=4.0)          # r4 = 4r
                TT(out=y[:, 2], in0=oA, in1=oC, op=ADD)   # y2 = p+4r
                TT(out=oA, in0=m1, in1=m2, op=SUB)        # q = m1-m2
                TT(out=oB, in0=m3, in1=m4, op=SUB)        # s = m3-m4
                TSM(out=oC, in0=oB, scalar1=2.0)          # s2 = 2s
                TT(out=y[:, 1], in0=oA, in1=oC, op=ADD)   # y1 = q+2s
                TSM(out=oB, in0=oC, scalar1=4.0)          # s8 = 8s
                TT(out=oC, in0=oA, in1=m5, op=ADD)        # u = q+m5
                TT(out=y[:, 3], in0=oC, in1=oB, op=ADD)   # y3 = u+8s

                # epilogue: prelu(scale*y + bias*G) -> f32 -> HBM
                for pair in range(2):
                    ob = obpool.tile([128, 2 * SS], F32, tag="ob",
                                     name=f"ob{z}_{pair}")
                    for j2 in range(2):
                        nc.scalar.activation(out=ob[:, j2 * SS:(j2 + 1) * SS],
                                             in_=y[:, 2 * pair + j2, :],
                                             func=AF.Prelu, bias=bias_g,
                                             scale=scale_sb, alpha=SLOPE)
                    d0 = 4 * z + 2 * pair
                    nc.sync.dma_start(out=out_d[:, d0 * SS:(d0 + 2) * SS],
                                      in_=ob)

    nc.compile()
    return nc


def kernel(**inputs):
    x = np.ascontiguousarray(np.asarray(inputs["x"], dtype=np.float32))
    w = np.ascontiguousarray(np.asarray(inputs["w"], dtype=np.float32))
    weight = np.ascontiguousarray(np.asarray(inputs["weight"], dtype=np.float32))
    aw = np.ascontiguousarray(np.asarray(inputs["affine_weight"], dtype=np.float32))
    ab = np.ascontiguousarray(np.asarray(inputs["affine_bias"], dtype=np.float32))
    bias = np.ascontiguousarray(np.asarray(inputs["bias"], dtype=np.float32))

    if "nc" not in _cache:
        _cache["nc"] = _build()
    nc = _cache["nc"]

    wt2 = weight.reshape(COUT, CIN * NTAPS)
    in_maps = [
        {
            "x": x[b].reshape(CIN, R * R * R),
            "wvec": w[b],
            "weight": wt2,
            "aw": aw,
            "ab": ab,
            "bias": bias,
        }
        for b in range(B)
    ]
    res = run_bass_kernel_spmd(nc, in_maps, list(range(NCORES)))
    out = np.stack([res.results[b]["out"].reshape(COUT, R, R, R) for b in range(B)])
    return out.astype(np.float32)


def run_traced(**inputs):
    """Like kernel(), but also returns the profiled HW exec time in ns."""
    x = np.asarray(inputs["x"], dtype=np.float32)
    w = np.asarray(inputs["w"], dtype=np.float32)
    weight = np.asarray(inputs["weight"], dtype=np.float32)
    aw = np.asarray(inputs["affine_weight"], dtype=np.float32)
    ab = np.asarray(inputs["affine_bias"], dtype=np.float32)
    bias = np.asarray(inputs["bias"], dtype=np.float32)
    if "nc" not in _cache:
        _cache["nc"] = _build()
    nc = _cache["nc"]
    wt2 = weight.reshape(COUT, CIN * NTAPS)
    in_maps = [
        {"x": x[b].reshape(CIN, R * R * R), "wvec": w[b], "weight": wt2,
         "aw": aw, "ab": ab, "bias": bias}
        for b in range(B)
    ]
    res = run_bass_kernel_spmd(nc, in_maps, list(range(NCORES)), trace=True)
    out = np.stack([res.results[b]["out"].reshape(COUT, R, R, R) for b in range(B)])
    return out.astype(np.float32), res.exec_time_ns, res
